# revision 2
# baseline (speedup 1.0000x reference)
# Trainium2 Bass kernel for nn_ContinuousHopfieldNet_70652212019686.
#
# Math (verified numerically against the jax reference):
#   B[i,:] = (k[4i] + k[4i+1] + k[4i+2] + k[4i+3]) / 4.5        (nb x d)
#   per retrieval iteration:
#     S = q @ B.T ; m = max(rowmax(S), 0) ; E = exp(S - m)
#     Z = E @ wbin + w_none * exp(-m) ; q' = (E @ (wbin*B)) / Z
#
# v2 sharding: the measured harness exec window is dominated by INPUT UPLOAD
# (baseline replicated the 16MB k to all 8 cores: 132.6MB ~ 3.1ms at PCIe
# ~43GB/s, while the device body is only ~110us).  So k is sharded: core c
# uploads rows [512c, 512c+512) (2MB), builds ITS 128-bin slice of the basis
# (Bw bf16, BT split-bf16 transposed), and one DRAM AllGather (6MB) gives
# every core the full basis.  Upload total drops to ~20.2MB -- the floor
# without moving input-dependent math to the host.
#
# Precision plan (inherited from v1, validated):
#   - iter-1 S: 3-term split-bf16 (Qh@BTh + Ql@BTh + Qh@BTl).
#   - iter-2/3 S: plain bf16; U = E@Bw plain bf16; E plain bf16; Z uses the
#     same truncated E so the leading-order E error cancels in U/Z.
#   - iter-1 needs NO max subtraction (raw scores <= ~324, exp(s/4.5) fits).
#   - basis matmuls use RAW binsums; the 1/4.5 rides the exp's scale.
#   - iters hand over U TRANSPOSED (= next S's lhsT layout) and UNNORMALIZED
#     (1/Z rides the next exp's per-partition scale).
import numpy as np

NB = 1024
D = 1024
KLEN = 4096
NQ = 1024
NPTS = 2048
NCORES = 8
QS = NQ // NCORES
KS = KLEN // NCORES
NITER = 3

MM_DTYPE = "bf16-plan-v2-allgather"  # informational


def _host_constants():
    """Input-independent basis constants, replicating reference fp32 math.

    Verified bit-identical to the jax reference in test.py."""
    t = np.linspace(0.0, 1.0, NPTS).astype(np.float32)
    dt = np.diff(t)
    w = np.concatenate([dt[:1] / 2, (dt[:-1] + dt[1:]) / 2, dt[-1:] / 2]).astype(
        np.float32
    )
    edges = (np.arange(NB + 1, dtype=np.float64) / NB).astype(np.float32)
    lb, ub = edges[:-1], edges[1:]
    cand = np.clip(np.searchsorted(ub, t, side="right"), 0, NB - 1)
    ok = (t >= lb[cand]) & (t < ub[cand])
    wbin64 = np.zeros(NB)
    np.add.at(wbin64, cand[ok], w[ok].astype(np.float64))
    wbin = wbin64.astype(np.float32)
    w_none = float(w[~ok].astype(np.float64).sum())
    # [128, 8] per-(partition, bin-chunk) layouts: wzc[p, c] = wbin[128c + p]
    wzc = wbin.reshape(8, 128).T.copy()
    wdiv = (wzc * np.float32(1.0 / 4.5)).astype(np.float32)
    wz = np.zeros((128, 8, 2), np.float32)  # N=2 pad for the Z matmul
    wz[:, :, 0] = wzc
    return wz, wdiv, w_none


def _build_program(bench_trips=0, bench_scope="full", ks_internal=False):
    import concourse.bacc as bacc
    import concourse.tile as tile
    from concourse import mybir
    from concourse.masks import make_identity

    F32 = mybir.dt.float32
    BF16 = mybir.dt.bfloat16
    SC = float(1.0 / 4.5)

    _, _, w_none = _host_constants()
    ln_wnone = float(np.log(np.float64(w_none)))

    nc = bacc.Bacc(
        "TRN2",
        target_bir_lowering=False,
        debug=False,
        enable_asserts=True,
        num_devices=NCORES,
    )
    ks_kind = "Internal" if ks_internal else "ExternalInput"
    ks = nc.dram_tensor("ks", [KS, D], F32, kind=ks_kind).ap()
    qs = nc.dram_tensor("qs", [QS, D], F32, kind="ExternalInput").ap()
    wz_d = nc.dram_tensor("wz", [128, 8, 2], F32, kind="ExternalInput").ap()
    wdivme_d = nc.dram_tensor("wdivme", [128, 1], F32, kind="ExternalInput").ap()
    out_d = nc.dram_tensor("out", [QS, D], F32, kind="ExternalOutput").ap()
    # collective payload: [0]=Bw chunk (bf16, [bin, d]); [1]/[2]=BT chunk
    # hi/lo (bf16, [d%128, (d//128)*128 + local_bin])
    payload = nc.dram_tensor("payload", [3, 128, D], BF16, kind="Internal").ap()
    gathered = nc.dram_tensor(
        "gathered", [3 * NCORES, 128, D], BF16, kind="Internal", addr_space="Shared"
    ).ap()

    with tile.TileContext(nc) as tc:
        with (
            tc.tile_pool(name="const", bufs=1) as constp,
            tc.tile_pool(name="ksrc", bufs=1) as kpool,
            tc.tile_pool(name="work", bufs=2) as work,
            tc.tile_pool(name="iterp", bufs=2) as iterp,
            tc.tile_pool(name="stats", bufs=4) as stats,
            tc.tile_pool(name="psA", bufs=1, space="PSUM") as psA,  # U/UT accum
            tc.tile_pool(name="psB", bufs=1, space="PSUM") as psB,  # S
            tc.tile_pool(name="psT", bufs=1, space="PSUM") as psT,  # f32 transposes
            tc.tile_pool(name="psTb", bufs=2, space="PSUM") as psTb,  # bf16 transposes
            tc.tile_pool(name="psZ", bufs=1, space="PSUM") as psZ,  # Z accum
        ):
            ident = constp.tile([128, 128], F32)
            make_identity(nc, ident)
            ident_bf = constp.tile([128, 128], BF16)
            nc.vector.tensor_copy(ident_bf, ident)
            lnw_sb = constp.tile([128, 1], F32)
            nc.vector.memset(lnw_sb, ln_wnone)
            wn_sb = constp.tile([128, 1], F32)
            nc.vector.memset(wn_sb, w_none)
            wz_sb = constp.tile([128, 8, 2], F32)
            nc.sync.dma_start(wz_sb, wz_d)
            wdivme_sb = constp.tile([128, 1], F32)
            nc.sync.dma_start(wdivme_sb, wdivme_d)
            wz_hi = constp.tile([128, 8, 2], BF16)
            nc.vector.tensor_copy(wz_hi, wz_sb)
            wz_lo = constp.tile([128, 8, 2], BF16)
            nc.vector.tensor_tensor(wz_lo, wz_sb, wz_hi, mybir.AluOpType.subtract)

            # full-basis weights, persistent across iterations (gathered)
            Bw_hi = constp.tile([128, 8, D], BF16, tag="Bw_hi")
            BT_hi = constp.tile([128, 8, 8, 128], BF16, tag="BT_hi")  # [p,c,kd,b]
            BT_lo = constp.tile([128, 8, 8, 128], BF16, tag="BT_lo")

            def build_basis_and_gather():
                """Build THIS core's 128-bin basis slice; AllGather the full
                basis (bf16 payload) into SBUF tiles Bw_hi/BT_hi/BT_lo."""
                kt = kpool.tile([128, 4, D], F32, tag="kt")
                nc.sync.dma_start(kt, ks.rearrange("(p r) d -> p r d", r=4))
                # binsum for the Bw path
                a1 = work.tile([128, D], F32, tag="a1")
                nc.vector.tensor_add(a1, kt[:, 0], kt[:, 1])
                a2 = work.tile([128, D], F32, tag="a2")
                nc.gpsimd.tensor_add(a2, kt[:, 2], kt[:, 3])
                bsum = work.tile([128, D], F32, tag="bsum")
                nc.vector.tensor_add(bsum, a1, a2)
                bw_own = work.tile([128, D], BF16, tag="bw_own")
                nc.scalar.mul(bw_own, bsum, wdivme_sb)
                nc.sync.dma_start(payload[0], bw_own)
                # BT chunk: 4-way accumulate-transpose straight from the k
                # tile (binsum.T without waiting for the vector adds)
                bt_own_hi = work.tile([128, 8, 128], BF16, tag="bt_own_hi")
                bt_own_lo = work.tile([128, 8, 128], BF16, tag="bt_own_lo")
                for h in range(2):
                    pt4 = psT.tile([128, 512], F32, tag="pt4")
                    for j in range(4):
                        kd = 4 * h + j
                        for r in range(4):
                            nc.tensor.matmul(
                                pt4[:, 128 * j : 128 * (j + 1)],
                                kt[:, r, 128 * kd : 128 * (kd + 1)],
                                ident,
                                is_transpose=True,
                                start=(r == 0),
                                stop=(r == 3),
                            )
                    pv = pt4.rearrange("p (a b) -> p a b", a=4)
                    nc.scalar.copy(bt_own_hi[:, 4 * h : 4 * h + 4, :], pv)
                    nc.vector.tensor_tensor(
                        bt_own_lo[:, 4 * h : 4 * h + 4, :],
                        pv,
                        bt_own_hi[:, 4 * h : 4 * h + 4, :],
                        mybir.AluOpType.subtract,
                    )
                nc.sync.dma_start(
                    payload[1].rearrange("p (kd b) -> p kd b", kd=8), bt_own_hi
                )
                nc.sync.dma_start(
                    payload[2].rearrange("p (kd b) -> p kd b", kd=8), bt_own_lo
                )
                nc.gpsimd.collective_compute(
                    "AllGather",
                    mybir.AluOpType.bypass,
                    replica_groups=[list(range(NCORES))],
                    ins=[payload],
                    outs=[gathered],
                )
                # unpack the gathered basis into SBUF
                gv = gathered.rearrange("(c s) p d -> s p c d", s=3)
                nc.sync.dma_start(Bw_hi, gv[0])
                gb = gathered.rearrange("(c s) p (kd b) -> s p c kd b", s=3, kd=8)
                nc.sync.dma_start(BT_hi, gb[1])
                nc.sync.dma_start(BT_lo, gb[2])

            def build_q0():
                """Qt1 hi/lo: Qt[p, kd, j] = q[j, 128 kd + p], split bf16."""
                qn = work.tile([128, D], F32, tag="qn")
                nc.sync.dma_start(qn, qs)
                Qt_hi = iterp.tile([128, 8, QS], BF16, tag="qt_hi")
                Qt_lo = iterp.tile([128, 8, QS], BF16, tag="qt_lo", name="qt_lo")
                for h in range(2):
                    pt4 = psT.tile([128, 512], F32, tag="pt4")
                    for j in range(4):
                        kd = 4 * h + j
                        nc.tensor.transpose(
                            pt4[:, 128 * j : 128 * (j + 1)],
                            qn[:, 128 * kd : 128 * (kd + 1)],
                            ident,
                        )
                    pv = pt4.rearrange("p (a b) -> p a b", a=4)
                    nc.scalar.copy(Qt_hi[:, 4 * h : 4 * h + 4, :], pv)
                    nc.vector.tensor_tensor(
                        Qt_lo[:, 4 * h : 4 * h + 4, :],
                        pv,
                        Qt_hi[:, 4 * h : 4 * h + 4, :],
                        mybir.AluOpType.subtract,
                    )
                return Qt_hi, Qt_lo

            def transpose_E(E, ET, blocks):
                """ET[:, c] = E[:, 128c:128(c+1)].T for c in blocks (bf16).
                blocks must be contiguous runs aligned to the ET layout."""
                for h in range(0, len(blocks), 4):
                    grp = blocks[h : h + 4]
                    ptb = psTb.tile([128, 512], BF16, tag="ptb")
                    for j, c in enumerate(grp):
                        nc.tensor.transpose(
                            ptb[:, 128 * j : 128 * (j + 1)],
                            E[:, 128 * c : 128 * (c + 1)],
                            ident_bf,
                        )
                    pv = ptb[:, : 128 * len(grp)].rearrange(
                        "p (a b) -> p a b", a=len(grp)
                    )
                    nc.vector.tensor_copy(ET[:, grp[0] : grp[0] + len(grp), :], pv)

            def accum_Z(Z, ET, c, first, last):
                nc.tensor.matmul(Z, ET[:, c], wz_hi[:, c], start=first, stop=False)
                nc.tensor.matmul(Z, ET[:, c], wz_lo[:, c], start=False, stop=last)

            def accum_U(U, ET, c, first, last):
                """U[q-part, d] += ET[:, c].T @ Bw[:, c] (one accumulation
                group per 512-wide PSUM bank region)."""
                for n in range(2):
                    ns = slice(512 * n, 512 * (n + 1))
                    nc.tensor.matmul(
                        U[:, ns], ET[:, c], Bw_hi[:, c, ns], start=first, stop=last
                    )

            def handover(U):
                """bf16 copy of the (unnormalized) U psum, transposed into
                the next iteration's lhsT layout."""
                qb = iterp.tile([128, D], BF16, tag="qb")
                nc.scalar.copy(qb, U)
                QtU = iterp.tile([128, 8, QS], BF16, tag="qt_hi")
                transpose_E(qb, QtU, list(range(8)))
                return QtU

            def iter1(Qt_hi, Qt_lo):
                """iter-1 (no-max softmax), chunk by chunk over bin blocks."""
                E1 = iterp.tile([128, NB], BF16, tag="E")
                ET1 = iterp.tile([128, 8, QS], BF16, tag="ET")
                U1 = psA.tile([128, D], F32, tag="U")
                S1 = psB.tile([128, NB], F32, tag="S")
                Z1 = psZ.tile([128, 2], F32, tag="Z")
                for c in range(8):
                    cs = slice(128 * c, 128 * (c + 1))
                    # 3-term split S for this bin block.  The two BT_hi terms
                    # are issued first so they can start earliest.
                    terms = [(Qt_hi, BT_hi), (Qt_lo, BT_hi), (Qt_hi, BT_lo)]
                    n_mm = len(terms) * 8
                    i_mm = 0
                    for lh, rh in terms:
                        for kd in range(8):
                            nc.tensor.matmul(
                                S1[:, cs],
                                lh[:, kd],
                                rh[:, c, kd],
                                start=(i_mm == 0),
                                stop=(i_mm == n_mm - 1),
                            )
                            i_mm += 1
                    # E (no max needed: raw scores <= ~324, exp(s/4.5) fits)
                    nc.scalar.activation(
                        E1[:, cs],
                        S1[:, cs],
                        mybir.ActivationFunctionType.Exp,
                        scale=SC,
                    )
                    transpose_E(E1, ET1, [c])
                    accum_Z(Z1, ET1, c, first=(c == 0), last=(c == 7))
                    accum_U(U1, ET1, c, first=(c == 0), last=(c == 7))
                # rc1 = 1 / (Z1 + w_none); handed to iter-2's exp as scale
                zf = stats.tile([128, 1], F32, tag="zf")
                nc.vector.tensor_add(zf, Z1[:, 0:1], wn_sb)
                rc = stats.tile([128, 1], F32, tag="rc")
                nc.vector.reciprocal(rc, zf)
                rcs = stats.tile([128, 1], F32, tag="rcs")
                nc.vector.tensor_scalar_mul(rcs, rc, SC)
                return handover(U1), rcs

            def iter23(QtU, rcs, last):
                """S from the unnormalized transposed U; 1/Z and 1/4.5 ride
                the exp scale. Returns (QtU', rcs') or writes the output."""
                S = psB.tile([128, NB], F32, tag="S")
                for c in range(8):
                    cs = slice(128 * c, 128 * (c + 1))
                    for kd in range(8):
                        nc.tensor.matmul(
                            S[:, cs],
                            QtU[:, kd],
                            BT_hi[:, c, kd],
                            start=(kd == 0),
                            stop=(kd == 7),
                        )
                # per-half row maxes so half-1's reduce overlaps half-2's S
                nmh = stats.tile([128, 2], F32, tag="nmh")
                for n in range(2):
                    ns = slice(512 * n, 512 * (n + 1))
                    nc.vector.reduce_max(
                        nmh[:, n : n + 1],
                        S[:, ns],
                        axis=mybir.AxisListType.X,
                        negate=True,
                    )
                nm = stats.tile([128, 1], F32, tag="nm")
                nc.vector.tensor_tensor(
                    nm, nmh[:, 0:1], nmh[:, 1:2], mybir.AluOpType.min
                )
                # negm = min(nm * rcs, 0)  [= -max(rowmax(S_true), 0) / 4.5]
                negm = stats.tile([128, 1], F32, tag="negm")
                nc.vector.tensor_scalar(
                    negm,
                    nm,
                    rcs,
                    0.0,
                    mybir.AluOpType.mult,
                    mybir.AluOpType.min,
                )
                E = iterp.tile([128, NB], BF16, tag="E")
                ET = iterp.tile([128, 8, QS], BF16, tag="ET")
                for n in range(2):
                    ns = slice(512 * n, 512 * (n + 1))
                    nc.scalar.activation(
                        E[:, ns],
                        S[:, ns],
                        mybir.ActivationFunctionType.Exp,
                        bias=negm,
                        scale=rcs,
                    )
                    transpose_E(E, ET, list(range(4 * n, 4 * n + 4)))
                Z = psZ.tile([128, 2], F32, tag="Z")
                for c in range(8):
                    accum_Z(Z, ET, c, first=(c == 0), last=(c == 7))
                # zc = w_none * exp(-m) = exp(4.5 * negm + ln w_none)
                zc = stats.tile([128, 1], F32, tag="zc")
                nc.scalar.activation(
                    zc,
                    negm,
                    mybir.ActivationFunctionType.Exp,
                    scale=4.5,
                    bias=lnw_sb[:, :1],
                )
                U = psA.tile([128, D], F32, tag="U")
                for c in range(8):
                    accum_U(U, ET, c, first=(c == 0), last=(c == 7))
                zf = stats.tile([128, 1], F32, tag="zf")
                nc.vector.tensor_add(zf, Z[:, 0:1], zc)
                rc = stats.tile([128, 1], F32, tag="rc")
                nc.vector.reciprocal(rc, zf)
                if last:
                    un = iterp.tile([128, D], F32, tag="un")
                    nc.scalar.mul(un, U, rc)
                    nc.sync.dma_start(out_d, un)
                    return None, None
                rcs2 = stats.tile([128, 1], F32, tag="rcs")
                nc.vector.tensor_scalar_mul(rcs2, rc, SC)
                return handover(U), rcs2

            def body():
                build_basis_and_gather()
                Qt_hi, Qt_lo = build_q0()
                QtU, rcs = iter1(Qt_hi, Qt_lo)
                QtU, rcs = iter23(QtU, rcs, last=False)
                iter23(QtU, rcs, last=True)

            if bench_trips and bench_scope == "build":
                with tc.For_i(0, bench_trips, 1):
                    build_basis_and_gather()
                    build_q0()
            elif bench_trips and bench_scope == "iters":
                build_basis_and_gather()
                Qt_hi, Qt_lo = build_q0()
                with tc.For_i(0, bench_trips, 1):
                    QtU, rcs = iter1(Qt_hi, Qt_lo)
                    QtU2, rcs2 = iter23(QtU, rcs, last=False)
                    iter23(QtU2, rcs2, last=True)
            elif bench_trips:
                with tc.For_i(0, bench_trips, 1):
                    body()
            else:
                body()

    nc.compile()
    return nc


_CACHE = {}
LAST_RESULTS = None


def kernel(**inputs):
    global LAST_RESULTS
    k = np.ascontiguousarray(np.asarray(inputs["k"], dtype=np.float32))
    q = np.ascontiguousarray(np.asarray(inputs["q"], dtype=np.float32))
    assert k.shape == (KLEN, D) and q.shape == (NQ, D)

    if "nc" not in _CACHE:
        _CACHE["nc"] = _build_program()
        _CACHE["consts"] = _host_constants()
    nc = _CACHE["nc"]
    wz, wdiv, _ = _CACHE["consts"]

    in_maps = []
    for c in range(NCORES):
        in_maps.append(
            {
                "ks": np.ascontiguousarray(k[KS * c : KS * (c + 1)]),
                "qs": np.ascontiguousarray(q[QS * c : QS * (c + 1)]),
                "wz": wz,
                "wdivme": np.ascontiguousarray(wdiv[:, c : c + 1]),
            }
        )

    import concourse.bass_utils as bass_utils

    res = bass_utils.run_bass_kernel_spmd(nc, in_maps, core_ids=list(range(NCORES)))
    LAST_RESULTS = res
    out = np.concatenate([res.results[c]["out"] for c in range(NCORES)], axis=0)
    return np.ascontiguousarray(out, dtype=np.float32)


if __name__ == "__main__":
    rng = np.random.default_rng(0)
    k = rng.standard_normal((KLEN, D), dtype=np.float32)
    q = rng.standard_normal((NQ, D), dtype=np.float32)
    o = kernel(k=k, q=q)
    print("kernel ran, out shape", o.shape, "finite:", np.isfinite(o).all())
